# revision 9
# baseline (speedup 1.0000x reference)
"""Mistral decoder layer (B=1, S=1024, HID=4096, 32 heads, INTER=11008), fp32,
tensor-parallel over 8 trn2 NeuronCores (Megatron style).

Everything on-chip runs in the "transposed" domain ([feature, seq] layouts) so
no on-chip transposes are needed anywhere:
  - host pre-transposes x, the mask, and all weight shards (and folds the RMS
    norm gains + 1/sqrt(hd) scale into the weights)
  - RMSNorm per-token sums-of-squares are computed with ones-matmuls on the
    TensorE (reduction over the partition dim), which also broadcasts the
    result to all 128 partitions for free
  - attention scores are computed transposed (ST[t, q]), softmax-exp applied
    elementwise, the PV product then contracts over t with expST as lhsT,
    producing attnT[hd, q] directly; rowsums come from ones-matmuls and the
    1/rowsum normalization is applied to attnT
  - residuals are folded into the AllReduces: each core contributes
    partial + residual/8, so the AR output is already residual-added
Matmuls run as float32r (TF32) at full PE rate; storage is fp32.
"""

import numpy as np

import concourse.bacc as bacc
import concourse.mybir as mybir
import concourse.tile as tile
from concourse.bass_utils import run_bass_kernel_spmd

AF = mybir.ActivationFunctionType
ALU = mybir.AluOpType
F32 = mybir.dt.float32
F32R = mybir.dt.float32r

N_CORES = 8
HID = 4096
S = 1024
NH = 32
HD = 128
NH_L = NH // N_CORES          # 4 local heads
DL = NH_L * HD                # 512 local q/k/v dims
INTER = 11008
IL_T = 11                     # local intermediate k-tiles (padded)
IL = IL_T * 128               # 1408 padded local intermediate
ILR = INTER // N_CORES        # 1376 real local intermediate
KT = HID // 128               # 32 hidden k-tiles
CH = 2                        # seq chunks
CW = S // CH                  # 512
TB = S // 128                 # 8 seq tiles of 128
EPS = 1e-5

_CACHE = {}


def _r(ap):
    return ap.bitcast(F32R)


def _build(collectives=True):
    nc = bacc.Bacc("TRN2", target_bir_lowering=False, debug=False,
                   num_devices=N_CORES)

    xT = nc.dram_tensor("xT", [HID, S], F32, kind="ExternalInput").ap()
    maskTd = nc.dram_tensor("maskTd", [TB, 128, CW], F32, kind="ExternalInput").ap()
    wqT = nc.dram_tensor("wqT", [HID, DL], F32, kind="ExternalInput").ap()
    wkT = nc.dram_tensor("wkT", [HID, DL], F32, kind="ExternalInput").ap()
    wvT = nc.dram_tensor("wvT", [HID, DL], F32, kind="ExternalInput").ap()
    woT = nc.dram_tensor("woT", [DL, HID], F32, kind="ExternalInput").ap()
    wuT = nc.dram_tensor("wuT", [HID, IL], F32, kind="ExternalInput").ap()
    wgT = nc.dram_tensor("wgT", [HID, IL], F32, kind="ExternalInput").ap()
    wdT = nc.dram_tensor("wdT", [IL, HID], F32, kind="ExternalInput").ap()
    outT = nc.dram_tensor("outT", [HID, S], F32, kind="ExternalOutput").ap()

    hT_d = nc.dram_tensor("hT_d", [HID, S], F32).ap()
    o_bounce = nc.dram_tensor("o_bounce", [HID, S], F32).ap()
    h2_d = nc.dram_tensor("h2_d", [HID, S], F32, addr_space="Shared").ap()
    m_d = nc.dram_tensor("m_d", [IL, S], F32).ap()
    dn_bounce = nc.dram_tensor("dn_bounce", [HID, S], F32).ap()
    dn_red = nc.dram_tensor("dn_red", [HID, S], F32, addr_space="Shared").ap()

    rg = [list(range(N_CORES))]

    with tile.TileContext(nc) as tc:
        with tc.tile_pool(name="const", bufs=1) as const:
            ones = const.tile([128, 128], F32, tag="ones")
            nc.vector.memset(ones[:], 1.0)
            s1 = const.tile([128, S], F32, tag="s1")
            s2 = const.tile([128, S], F32, tag="s2")
            epst = const.tile([128, 1], F32, tag="epst")
            nc.vector.memset(epst[:], EPS)

            # ---------- Phase 1: RMSNorm #1, hT = x * rsqrt(mean(x^2)+eps) ----------
            # (g_in and the attention 1/sqrt(hd) are folded into the weights)
            with (
                tc.tile_pool(name="p1", bufs=3) as p1,
                tc.tile_pool(name="p1m", bufs=2) as p1m,
                tc.tile_pool(name="p1ps", bufs=1, space="PSUM") as p1ps,
            ):
                r2 = [p1ps.tile([128, CW], F32, tag=f"r2_{c}", name=f"r2_{c}") for c in range(CH)]
                for k in range(KT):
                    xt = p1.tile([128, S], F32, tag="xt")
                    nc.sync.dma_start(xt[:], xT[k * 128:(k + 1) * 128, :])
                    sq = p1.tile([128, S], F32, tag="sq")
                    nc.scalar.activation(_r(sq[:]), xt[:], AF.Square)
                    for c in range(CH):
                        nc.tensor.matmul(r2[c][:], _r(ones[:]),
                                         _r(sq[:, c * CW:(c + 1) * CW]),
                                         start=(k == 0), stop=(k == KT - 1))
                for c in range(CH):
                    ms = p1m.tile([128, CW], F32, tag="ms")
                    # sqrt(r2/HID + EPS), then exact reciprocal
                    nc.scalar.activation(ms[:], r2[c][:], AF.Sqrt,
                                         bias=epst[:], scale=1.0 / HID)
                    nc.vector.reciprocal(s1[:, c * CW:(c + 1) * CW], ms[:])
                for k in range(KT):
                    xt = p1.tile([128, S], F32, tag="xt")
                    nc.sync.dma_start(xt[:], xT[k * 128:(k + 1) * 128, :])
                    ht = p1.tile([128, S], F32, tag="ht")
                    nc.vector.tensor_mul(_r(ht[:]), xt[:], s1[:])
                    nc.sync.dma_start(hT_d[k * 128:(k + 1) * 128, :], ht[:])

            # ---------- Phases 2-4: QKV, attention, o-proj ----------
            with tc.tile_pool(name="qkvo", bufs=1) as qkvo:
                QTt = [qkvo.tile([128, S], F32, tag=f"QT{h}", name=f"QT{h}") for h in range(NH_L)]
                KTt = [qkvo.tile([128, S], F32, tag=f"KT{h}", name=f"KT{h}") for h in range(NH_L)]
                Vt = [qkvo.tile([128, DL], F32, tag=f"V{t}", name=f"V{t}") for t in range(TB)]
                ATt = [qkvo.tile([128, S], F32, tag=f"AT{h}", name=f"AT{h}") for h in range(NH_L)]

                # --- q / k projections: out[h] = (w.T g x s1)^T per head, [hd, seq]
                for nm, wT, outs in (("q", wqT, QTt), ("k", wkT, KTt)):
                    with (
                        tc.tile_pool(name=f"{nm}w", bufs=3) as wp,
                        tc.tile_pool(name=f"{nm}h", bufs=3) as hp,
                        tc.tile_pool(name=f"{nm}ps", bufs=1, space="PSUM") as ps,
                    ):
                        pt = [ps.tile([128, CW], F32, tag=f"pt{j}", name=f"pt{j}")
                              for j in range(NH_L * CH)]
                        for k in range(KT):
                            wt = wp.tile([128, DL], F32, tag="wt")
                            nc.sync.dma_start(_r(wt[:]), _r(wT[k * 128:(k + 1) * 128, :]))
                            ht = hp.tile([128, S], F32, tag="ht")
                            nc.sync.dma_start(_r(ht[:]), _r(hT_d[k * 128:(k + 1) * 128, :]))
                            for h in range(NH_L):
                                for c in range(CH):
                                    nc.tensor.matmul(
                                        pt[h * CH + c][:],
                                        _r(wt[:, h * 128:(h + 1) * 128]),
                                        _r(ht[:, c * CW:(c + 1) * CW]),
                                        start=(k == 0), stop=(k == KT - 1))
                        for h in range(NH_L):
                            for c in range(CH):
                                nc.vector.tensor_copy(
                                    _r(outs[h][:, c * CW:(c + 1) * CW]),
                                    pt[h * CH + c][:])

                # --- v projection in [seq, hd] layout: V[t] = [128 t, DL]
                with (
                    tc.tile_pool(name="vw", bufs=3) as wp,
                    tc.tile_pool(name="vh", bufs=3) as hp,
                    tc.tile_pool(name="vps", bufs=1, space="PSUM") as ps,
                ):
                    pt = [ps.tile([128, DL], F32, tag=f"pt{t}", name=f"pt{t}") for t in range(TB)]
                    for k in range(KT):
                        wt = wp.tile([128, DL], F32, tag="wt")
                        nc.sync.dma_start(_r(wt[:]), _r(wvT[k * 128:(k + 1) * 128, :]))
                        ht = hp.tile([128, S], F32, tag="ht")
                        nc.sync.dma_start(_r(ht[:]), _r(hT_d[k * 128:(k + 1) * 128, :]))
                        for t in range(TB):
                            nc.tensor.matmul(
                                pt[t][:], _r(ht[:, t * 128:(t + 1) * 128]),
                                _r(wt[:]), start=(k == 0), stop=(k == KT - 1))
                    for t in range(TB):
                        nc.vector.tensor_copy(_r(Vt[t][:]), pt[t][:])

                # --- attention (causal): per head, per seq-chunk
                with (
                    tc.tile_pool(name="mask", bufs=1) as mp,
                    tc.tile_pool(name="est", bufs=2) as estp,
                    tc.tile_pool(name="rin", bufs=2) as rinp,
                    tc.tile_pool(name="aps", bufs=1, space="PSUM") as aps,
                    tc.tile_pool(name="stps", bufs=3, space="PSUM") as stps,
                ):
                    mtiles = []
                    for t in range(TB):
                        mt = mp.tile([128, CW], F32, tag=f"m{t}", name=f"mk{t}")
                        nc.sync.dma_start(mt[:], maskTd[t, :, :])
                        mtiles.append(mt)
                    atp = [aps.tile([128, CW], F32, tag=f"atp{j}", name=f"atp{j}") for j in range(2)]
                    rsp = [aps.tile([128, CW], F32, tag=f"rsp{j}", name=f"rsp{j}") for j in range(2)]
                    for h in range(NH_L):
                        for c in range(CH):
                            tbs = list(range(0, (c + 1) * 4))
                            ets = []
                            for t in tbs:
                                stp = stps.tile([128, CW], F32, tag="st")
                                nc.tensor.matmul(
                                    stp[:], _r(KTt[h][:, t * 128:(t + 1) * 128]),
                                    _r(QTt[h][:, c * CW:(c + 1) * CW]),
                                    start=True, stop=True)
                                et = estp.tile([128, CW], F32, tag=f"et{t}")
                                if t >= c * 4:  # diagonal tile: apply causal mask
                                    nc.vector.tensor_add(_r(et[:]), stp[:], mtiles[t][:])
                                    nc.scalar.activation(_r(et[:]), et[:], AF.Exp)
                                else:
                                    nc.scalar.activation(_r(et[:]), stp[:], AF.Exp)
                                ets.append(et)
                            ap_, rp_ = atp[c], rsp[c]
                            for j, t in enumerate(tbs):
                                st_, sp_ = (j == 0), (j == len(tbs) - 1)
                                nc.tensor.matmul(
                                    ap_[:], _r(Vt[t][:, h * 128:(h + 1) * 128]),
                                    _r(ets[j][:]), start=st_, stop=sp_)
                                nc.tensor.matmul(
                                    rp_[:], _r(ones[:]), _r(ets[j][:]),
                                    start=st_, stop=sp_)
                            ri = rinp.tile([128, CW], F32, tag="ri")
                            nc.vector.reciprocal(ri[:], rp_[:])
                            nc.vector.tensor_mul(
                                _r(ATt[h][:, c * CW:(c + 1) * CW]), ap_[:], ri[:])

                # --- o-proj partial + x/8 residual fold, then AllReduce #1
                with (
                    tc.tile_pool(name="ow", bufs=2) as owp,
                    tc.tile_pool(name="ox", bufs=3) as oxp,
                    tc.tile_pool(name="ops", bufs=2, space="PSUM") as ops,
                    tc.tile_pool(name="oev", bufs=3) as oev,
                ):
                    for mh in range(KT):
                        wts = []
                        for h in range(NH_L):
                            wt = owp.tile([128, 128], F32, tag=f"wt{h}")
                            nc.sync.dma_start(
                                _r(wt[:]), _r(woT[h * 128:(h + 1) * 128,
                                                  mh * 128:(mh + 1) * 128]))
                            wts.append(wt)
                        xt = oxp.tile([128, S], F32, tag="xt")
                        nc.sync.dma_start(xt[:], xT[mh * 128:(mh + 1) * 128, :])
                        for c in range(CH):
                            pt = ops.tile([128, CW], F32, tag="pt")
                            for h in range(NH_L):
                                nc.tensor.matmul(
                                    pt[:], _r(wts[h][:]),
                                    _r(ATt[h][:, c * CW:(c + 1) * CW]),
                                    start=(h == 0), stop=(h == NH_L - 1))
                            ev = oev.tile([128, CW], F32, tag="ev")
                            nc.vector.scalar_tensor_tensor(
                                ev[:], xt[:, c * CW:(c + 1) * CW],
                                1.0 / N_CORES, pt[:], op0=ALU.mult, op1=ALU.add)
                            nc.sync.dma_start(
                                o_bounce[mh * 128:(mh + 1) * 128,
                                         c * CW:(c + 1) * CW], ev[:])

            if collectives:
                nc.gpsimd.collective_compute(
                    "AllReduce", ALU.add, ins=[o_bounce[:]], outs=[h2_d[:]],
                    replica_groups=rg)
            else:
                nc.gpsimd.dma_start(h2_d[:], o_bounce[:])

            # ---------- Phase 5: RMSNorm #2 (h2 -> n2, kept resident) ----------
            with tc.tile_pool(name="n2", bufs=1) as n2pool:
                n2t = [n2pool.tile([128, S], F32, tag=f"n2_{k}", name=f"n2_{k}") for k in range(KT)]
                with (
                    tc.tile_pool(name="p5", bufs=3) as p5,
                    tc.tile_pool(name="p5m", bufs=2) as p5m,
                    tc.tile_pool(name="p5ps", bufs=1, space="PSUM") as p5ps,
                ):
                    r2 = [p5ps.tile([128, CW], F32, tag=f"r2_{c}", name=f"r2b_{c}") for c in range(CH)]
                    for k in range(KT):
                        nc.sync.dma_start(n2t[k][:], h2_d[k * 128:(k + 1) * 128, :])
                        sq = p5.tile([128, S], F32, tag="sq")
                        nc.scalar.activation(_r(sq[:]), n2t[k][:], AF.Square)
                        for c in range(CH):
                            nc.tensor.matmul(r2[c][:], _r(ones[:]),
                                             _r(sq[:, c * CW:(c + 1) * CW]),
                                             start=(k == 0), stop=(k == KT - 1))
                    for c in range(CH):
                        ms = p5m.tile([128, CW], F32, tag="ms")
                        nc.scalar.activation(ms[:], r2[c][:], AF.Sqrt,
                                             bias=epst[:], scale=1.0 / HID)
                        nc.vector.reciprocal(s2[:, c * CW:(c + 1) * CW], ms[:])
                    for k in range(KT):
                        nc.vector.tensor_mul(_r(n2t[k][:]), n2t[k][:], s2[:])

                # ---------- Phase 6: up/gate + silu-mul, m -> DRAM ----------
                # (g_post folded into wuT/wgT)
                with (
                    tc.tile_pool(name="ugw", bufs=1) as ugw,
                    tc.tile_pool(name="ugps", bufs=2, space="PSUM") as ugps,
                    tc.tile_pool(name="ugt", bufs=3) as ugt,
                ):
                    HK = KT // 2
                    for d in range(IL_T):
                        slabs = {}
                        for nm, wT in (("u", wuT), ("g", wgT)):
                            halves = []
                            for hv in range(2):
                                sl = ugw.tile([128, HK * 128], F32, tag=f"{nm}{hv}", name=f"slab_{nm}{hv}")
                                src = wT[hv * HK * 128:(hv + 1) * HK * 128,
                                         d * 128:(d + 1) * 128]
                                nc.sync.dma_start(
                                    _r(sl[:].rearrange("p (kt d) -> p kt d", kt=HK)),
                                    _r(src.rearrange("(kt p) d -> p kt d", p=128)))
                                halves.append(sl)
                            slabs[nm] = halves
                        pts = {}
                        for nm in ("u", "g"):
                            for c in range(CH):
                                pt = ugps.tile([128, CW], F32, tag=f"pt{nm}{c}", name=f"pt{nm}{c}")
                                for k in range(KT):
                                    sl = slabs[nm][k // HK]
                                    kk = k % HK
                                    nc.tensor.matmul(
                                        pt[:], _r(sl[:, kk * 128:(kk + 1) * 128]),
                                        _r(n2t[k][:, c * CW:(c + 1) * CW]),
                                        start=(k == 0), stop=(k == KT - 1))
                                pts[(nm, c)] = pt
                        for c in range(CH):
                            sil = ugt.tile([128, CW], F32, tag="sil")
                            nc.scalar.activation(sil[:], pts[("u", c)][:], AF.Silu)
                            mt = ugt.tile([128, CW], F32, tag="mt")
                            nc.vector.tensor_mul(mt[:], sil[:], pts[("g", c)][:])
                            nc.sync.dma_start(
                                m_d[d * 128:(d + 1) * 128, c * CW:(c + 1) * CW],
                                mt[:])

            # ---------- Phase 7: down-proj partial + h2/8 fold, AllReduce #2 ----
            with (
                tc.tile_pool(name="mres", bufs=1) as mres,
                tc.tile_pool(name="dw", bufs=2) as dwp,
                tc.tile_pool(name="dh", bufs=3) as dhp,
                tc.tile_pool(name="dps", bufs=2, space="PSUM") as dps,
                tc.tile_pool(name="dev", bufs=3) as dev,
            ):
                m_t = [mres.tile([128, S], F32, tag=f"m{i}", name=f"mres{i}") for i in range(IL_T)]
                for i in range(IL_T):
                    nc.sync.dma_start(_r(m_t[i][:]), _r(m_d[i * 128:(i + 1) * 128, :]))
                for mh in range(KT):
                    sl = dwp.tile([128, IL_T * 128], F32, tag="dw")
                    src = wdT[:, mh * 128:(mh + 1) * 128]
                    nc.sync.dma_start(
                        _r(sl[:].rearrange("p (it d) -> p it d", it=IL_T)),
                        _r(src.rearrange("(it p) d -> p it d", p=128)))
                    h2t = dhp.tile([128, S], F32, tag="h2t")
                    nc.sync.dma_start(h2t[:], h2_d[mh * 128:(mh + 1) * 128, :])
                    for c in range(CH):
                        pt = dps.tile([128, CW], F32, tag="pt")
                        for i in range(IL_T):
                            nc.tensor.matmul(
                                pt[:], _r(sl[:, i * 128:(i + 1) * 128]),
                                _r(m_t[i][:, c * CW:(c + 1) * CW]),
                                start=(i == 0), stop=(i == IL_T - 1))
                        ev = dev.tile([128, CW], F32, tag="ev")
                        nc.vector.scalar_tensor_tensor(
                            ev[:], h2t[:, c * CW:(c + 1) * CW],
                            1.0 / N_CORES, pt[:], op0=ALU.mult, op1=ALU.add)
                        nc.sync.dma_start(
                            dn_bounce[mh * 128:(mh + 1) * 128,
                                      c * CW:(c + 1) * CW], ev[:])

            if collectives:
                nc.gpsimd.collective_compute(
                    "AllReduce", ALU.add, ins=[dn_bounce[:]], outs=[dn_red[:]],
                    replica_groups=rg)
            else:
                nc.gpsimd.dma_start(dn_red[:], dn_bounce[:])
            nc.sync.dma_start(outT[:], dn_red[:])

    nc.compile()
    return nc


def _host_shard(hidden_states, mask, wq, wk, wv, wo, w_gate, w_up, w_down,
                g_in, g_post):
    x = np.asarray(hidden_states, dtype=np.float32).reshape(S, HID)
    xT = np.ascontiguousarray(x.T)
    maskT = np.ascontiguousarray(np.asarray(mask, dtype=np.float32)
                                 .reshape(S, S).T)
    maskTd = np.empty((TB, 128, CW), np.float32)
    for t in range(TB):
        c = t // (TB // CH)
        maskTd[t] = maskT[t * 128:(t + 1) * 128, c * CW:(c + 1) * CW]
    g_in = np.asarray(g_in, dtype=np.float32)
    g_post = np.asarray(g_post, dtype=np.float32)
    sc = np.float32(HD ** -0.5)

    in_maps = []
    for i in range(N_CORES):
        r0, r1 = i * DL, (i + 1) * DL
        i0, i1 = i * ILR, (i + 1) * ILR
        wqT = np.ascontiguousarray(wq[r0:r1].T * (g_in[:, None] * sc))
        wkT = np.ascontiguousarray(wk[r0:r1].T * g_in[:, None])
        wvT = np.ascontiguousarray(wv[r0:r1].T * g_in[:, None])
        woT = np.ascontiguousarray(wo[:, r0:r1].T)
        wuT = np.zeros((HID, IL), np.float32)
        wuT[:, :ILR] = w_up[i0:i1].T * g_post[:, None]
        wgT = np.zeros((HID, IL), np.float32)
        wgT[:, :ILR] = w_gate[i0:i1].T * g_post[:, None]
        wdT = np.zeros((IL, HID), np.float32)
        wdT[:ILR] = w_down[:, i0:i1].T
        in_maps.append({
            "xT": xT, "maskTd": maskTd, "wqT": wqT, "wkT": wkT, "wvT": wvT,
            "woT": woT, "wuT": wuT, "wgT": wgT, "wdT": wdT,
        })
    return in_maps


def _get_nc():
    if "nc" not in _CACHE:
        _CACHE["nc"] = _build()
    return _CACHE["nc"]


def kernel(**inputs):
    nc = _get_nc()
    in_maps = _host_shard(**{k: np.asarray(v) for k, v in inputs.items()})
    res = run_bass_kernel_spmd(nc, in_maps, list(range(N_CORES)))
    outT = res.results[0]["outT"]
    return np.ascontiguousarray(outT.T).reshape(1, S, HID)


def bench(iters=8, **inputs):
    """Time repeated on-device executions of the compiled kernel; returns
    (best_ns, outputs_of_last_run_core0)."""
    import time
    import jax
    from jax.sharding import Mesh, PartitionSpec
    from jax.experimental.shard_map import shard_map
    from concourse import bass2jax

    nc = _get_nc()
    in_maps = _host_shard(**{k: np.asarray(v) for k, v in inputs.items()})
    bass2jax.install_neuronx_cc_hook()

    partition_name = (nc.partition_id_tensor.name
                      if nc.partition_id_tensor else None)
    in_names, out_names, out_avals, zero_outs = [], [], [], []
    for alloc in nc.m.functions[0].allocations:
        if not isinstance(alloc, mybir.MemoryLocationSet):
            continue
        name = alloc.memorylocations[0].name
        if alloc.kind == "ExternalInput":
            if name != partition_name:
                in_names.append(name)
        elif alloc.kind == "ExternalOutput":
            out_names.append(name)
            shape = tuple(alloc.tensor_shape)
            dtype = mybir.dt.np(alloc.dtype)
            out_avals.append(jax.core.ShapedArray(shape, dtype))
            zero_outs.append(np.zeros(shape, dtype))
    n_params = len(in_names)
    all_in = list(in_names) + list(out_names)
    if partition_name is not None:
        all_in.append(partition_name)

    def _body(*args):
        operands = list(args)
        if partition_name is not None:
            operands.append(bass2jax.partition_id_tensor())
        outs = bass2jax._bass_exec_p.bind(
            *operands,
            out_avals=tuple(out_avals), in_names=tuple(all_in),
            out_names=tuple(out_names), lowering_input_output_aliases=(),
            sim_require_finite=True, sim_require_nnan=True, nc=nc)
        return tuple(outs)

    devices = jax.devices()[:N_CORES]
    mesh = Mesh(np.asarray(devices), ("core",))
    n_outs = len(out_names)
    in_specs = (PartitionSpec("core"),) * (n_params + n_outs)
    out_specs = (PartitionSpec("core"),) * n_outs
    fn = jax.jit(shard_map(_body, mesh=mesh, in_specs=in_specs,
                           out_specs=out_specs, check_rep=False))
    concat_in = [np.concatenate([np.asarray(in_maps[c][nm])
                                 for c in range(N_CORES)], axis=0)
                 for nm in in_names]
    concat_zeros = [np.zeros((N_CORES * z.shape[0], *z.shape[1:]), z.dtype)
                    for z in zero_outs]
    sharding = jax.sharding.NamedSharding(mesh, PartitionSpec("core"))
    dev_in = [jax.device_put(a, sharding) for a in concat_in]
    dev_zero = [jax.device_put(a, sharding) for a in concat_zeros]

    outs = fn(*dev_in, *dev_zero)          # warm-up / compile
    jax.block_until_ready(outs)
    best = float("inf")
    for _ in range(iters):
        t0 = time.perf_counter_ns()
        outs = fn(*dev_in, *dev_zero)
        jax.block_until_ready(outs)
        best = min(best, time.perf_counter_ns() - t0)
    res0 = {nm: np.asarray(outs[i]).reshape(N_CORES, *out_avals[i].shape)[0]
            for i, nm in enumerate(out_names)}
    return best, res0
